# revision 8
# baseline (speedup 1.0000x reference)
"""Causal self-attention Trainium2 kernel (B=4, T=2048, E=1024, H=16, D=64).

Sharding: 8 cores = batch(4) x head-group(2). Each core computes the full
attention for 8 heads of one batch element plus its half of the output
projection; the host sums the two out-proj partials per batch element.

Dataflow (per core, all matmuls in float32r at full PE rate):
  - Host pre-transposes x and the weights so contraction dims are on
    partitions: xT [E,T], wqkvT [E,1536], woT [512,E].
  - QKV projection produces Q^T/K^T in [d,T] layout (head pairs packed into
    128 partitions) and V in natural [T,d] layout with an appended ones
    column (softmax denominator rides through the PV matmul).
  - Attention per (head-pair, 512-query tile): S^T chunks [128kv, 512q] via
    row-packed K=64 matmul pairs -> exp on ScalarE (scale=1/sqrt(D)) ->
    causal mask multiply on VectorE (diagonal chunks only) ->
    yT accumulation [65, 512] with lhsT = V_aug.
  - Normalize: reciprocal of the ones-row, gpsimd partition-broadcast,
    VectorE multiply into yT [f, T] layout.
  - Out-proj: natural-layout [T,E] partial via lhsT = yT slices.
"""

import os
import numpy as np

import concourse.bass as bass
import concourse.bacc as bacc
import concourse.mybir as mybir
import concourse.tile as tile
from concourse import bass_utils

f32 = mybir.dt.float32
f32r = mybir.dt.float32r
FP = mybir.dt.float32  # psum dtype

P = 128
B, T, E = 4, 2048, 1024
H, D = 16, 64
HPC = H // 2            # heads per core = 8
NE = E // P             # 8 e-chunks
NTT = T // P            # 16 kv chunks
NQ = T // 512           # 4 query tiles of 512
SCALE = 1.0 / np.sqrt(D)

Exp = mybir.ActivationFunctionType.Exp
MULT = mybir.AluOpType.mult

_CACHE = {}


def build(reps=1):
    nc = bacc.Bacc("TRN2", target_bir_lowering=False, debug=False, num_devices=8)

    xT_d = nc.dram_tensor("xT", [E, T], f32r, kind="ExternalInput")
    wqkvT_d = nc.dram_tensor("wqkvT", [E, 3 * 512], f32r, kind="ExternalInput")
    woT_d = nc.dram_tensor("woT", [512, E], f32r, kind="ExternalInput")
    masks_d = nc.dram_tensor("masks", [P, 4, 512], f32, kind="ExternalInput")
    ones_d = nc.dram_tensor("vones", [P, NTT, HPC, 1], f32r, kind="ExternalInput")
    out_d = nc.dram_tensor("out", [T, E], f32, kind="ExternalOutput")

    with tile.TileContext(nc) as tc:
        for rep in range(reps):
            build_body(tc, xT_d, wqkvT_d, woT_d, masks_d, ones_d, out_d, rep)
    nc.compile()
    return nc


def build_body(tc, xT_d, wqkvT_d, woT_d, masks_d, ones_d, out_d, rep=0):
    nc = tc.nc

    from contextlib import ExitStack
    with ExitStack() as top:
        cst = top.enter_context(tc.tile_pool(name="cst", bufs=1))
        per = top.enter_context(tc.tile_pool(name="per", bufs=1))

        masks_sb = cst.tile([P, 4, 512], f32)
        nc.sync.dma_start(masks_sb, masks_d[:, :, :])

        qk_sb = per.tile([P, 8, T], f32r)           # chunks 0-3: Q^T, 4-7: K^T
        v_sb = per.tile([P, NTT, HPC, D + 1], f32r)  # [kv_p, kv_chunk, head, d|1]
        yt_sb = per.tile([P, 4, T], f32r)            # [f%128, f//128, q]

        # ones column for the softmax denominator (DMA'd: memset can't emit f32r)
        nc.sync.dma_start(v_sb[:, :, :, D:D + 1], ones_d[:, :, :, :])

        # ---------------- Projection phase (two T-halves) ----------------
        with ExitStack() as proj:
            wpool = proj.enter_context(tc.tile_pool(name="wpool", bufs=2))
            xpool = proj.enter_context(tc.tile_pool(name="xpool", bufs=1))
            psp = proj.enter_context(tc.tile_pool(name="psp", bufs=2, space="PSUM"))

            wv_sb = wpool.tile([P, NE, 512], f32r, tag="wv", bufs=1)
            nc.sync.dma_start(
                wv_sb, wqkvT_d[:, 1024:1536].rearrange("(o p) f -> p o f", p=P))

            for th in range(2):
                xts = []
                for e in range(NE):
                    xt = xpool.tile([P, 1024], f32r, tag=f"xt{e}")
                    nc.sync.dma_start(
                        xt, xT_d[e * P:(e + 1) * P, th * 1024:(th + 1) * 1024])
                    xts.append(xt)

                # V projection: natural layout [T, 512]
                for tti in range(8):
                    tt = th * 8 + tti
                    ps = psp.tile([P, 512], FP, tag="psv")
                    for e in range(NE):
                        nc.tensor.matmul(
                            ps,
                            lhsT=xts[e][:, tti * P:(tti + 1) * P],
                            rhs=wv_sb[:, e, :],
                            start=(e == 0), stop=(e == NE - 1))
                    nc.vector.tensor_copy(
                        v_sb[:, tt, :, 0:D],
                        ps.rearrange("p (h d) -> p h d", h=HPC))

                # QK^T projection: [f, T] layout
                for ft in (0, 4, 1, 5, 2, 6, 3, 7):
                    wqk = wpool.tile([P, NE, P], f32r, tag="wq")
                    nc.sync.dma_start(
                        wqk,
                        wqkvT_d[:, ft * P:(ft + 1) * P].rearrange(
                            "(o p) f -> p o f", p=P))
                    for ts in range(2):
                        ps = psp.tile([P, 512], FP, tag="psqk")
                        for e in range(NE):
                            nc.tensor.matmul(
                                ps,
                                lhsT=wqk[:, e, :],
                                rhs=xts[e][:, ts * 512:(ts + 1) * 512],
                                start=(e == 0), stop=(e == NE - 1))
                        nc.vector.tensor_copy(
                            qk_sb[:, ft, th * 1024 + ts * 512:th * 1024 + (ts + 1) * 512],
                            ps)

        # ---------------- Attention phase ----------------
        with ExitStack() as att:
            ptp = att.enter_context(tc.tile_pool(name="ptp", bufs=4))
            nrm = att.enter_context(tc.tile_pool(name="nrm", bufs=2))
            pss = att.enter_context(tc.tile_pool(name="pss", bufs=2, space="PSUM"))
            psy = att.enter_context(tc.tile_pool(name="psy", bufs=3, space="PSUM"))

            for c in range(4):          # head pair (2c, 2c+1)
                for j in range(NQ):     # query tile of 512
                    nkv = 4 * j + 4
                    jsl = slice(j * 512, (j + 1) * 512)
                    yps = [psy.tile([D + 1, 512], FP, tag="y", name=f"yps{rep}_{c}_{j}_{k}")
                           for k in range(2)]
                    for i in range(nkv):
                        spt = pss.tile([P, 1024], FP, tag="s")
                        for hh in range(2):
                            p0 = 64 * hh
                            nc.tensor.matmul(
                                spt[:, hh * 512:(hh + 1) * 512],
                                lhsT=qk_sb[p0:p0 + 64, 4 + c, i * P:(i + 1) * P],
                                rhs=qk_sb[p0:p0 + 64, c, jsl],
                                start=True, stop=True)
                        ptt = ptp.tile([P, 1024], f32r, tag="pt")
                        nc.scalar.activation(ptt, spt, Exp, scale=float(SCALE))
                        off = i - 4 * j
                        if off >= 0:  # diagonal chunk: causal mask multiply
                            W = P * (off + 1)
                            for hh in range(2):
                                nc.vector.tensor_tensor(
                                    ptt[:, hh * 512:hh * 512 + W],
                                    ptt[:, hh * 512:hh * 512 + W],
                                    masks_sb[:, off, 0:W],
                                    MULT)
                        for hh in range(2):
                            nc.tensor.matmul(
                                yps[hh],
                                lhsT=v_sb[:, i, 2 * c + hh, :],
                                rhs=ptt[:, hh * 512:(hh + 1) * 512],
                                start=(i == 0), stop=(i == nkv - 1))
                    for hh in range(2):
                        rc = nrm.tile([1, 512], f32, tag="rc")
                        nc.vector.reciprocal(rc, yps[hh][D:D + 1, :])
                        bc = nrm.tile([64, 512], f32, tag="bc")
                        nc.gpsimd.partition_broadcast(bc, rc)
                        if hh == 0:
                            nc.vector.tensor_tensor(
                                yt_sb[0:64, c, jsl], yps[hh][0:D, :], bc, MULT)
                        else:
                            tmp = nrm.tile([64, 512], f32r, tag="tmp")
                            nc.vector.tensor_tensor(tmp, yps[hh][0:D, :], bc, MULT)
                            nc.sync.dma_start(yt_sb[64:128, c, jsl], tmp)

        # ---------------- Output projection ----------------
        with ExitStack() as op:
            wop = op.enter_context(tc.tile_pool(name="wop", bufs=1))
            ost = op.enter_context(tc.tile_pool(name="ost", bufs=3))
            pso = op.enter_context(tc.tile_pool(name="pso", bufs=2, space="PSUM"))

            wo_sb = wop.tile([P, 4, 1024], f32r)
            nc.sync.dma_start(wo_sb, woT_d.rearrange("(o p) f -> p o f", p=P))

            for tt in range(NTT):
                for half in range(2):
                    ps = pso.tile([P, 512], FP, tag="po")
                    for c2 in range(4):
                        nc.tensor.matmul(
                            ps,
                            lhsT=yt_sb[:, c2, tt * P:(tt + 1) * P],
                            rhs=wo_sb[:, c2, half * 512:(half + 1) * 512],
                            start=(c2 == 0), stop=(c2 == 3))
                    st = ost.tile([P, 512], f32, tag="st")
                    nc.vector.tensor_copy(st, ps)
                    nc.sync.dma_start(
                        out_d[tt * P:(tt + 1) * P, half * 512:(half + 1) * 512], st)


def _masks_np():
    p = np.arange(P)[:, None, None]
    off = np.arange(4)[None, :, None]
    q = np.arange(512)[None, None, :]
    return (128 * off + p <= q).astype(np.float32)


def _shard_inputs(x, w_qkv, w_out):
    masks = _masks_np()
    in_maps = []
    for core in range(8):
        b, hg = core // 2, core % 2
        sl = slice(hg * 512, (hg + 1) * 512)
        wq = w_qkv[0:1024][sl]
        wk = w_qkv[1024:2048][sl]
        wv = w_qkv[2048:3072][sl]
        wqkvT = np.ascontiguousarray(np.concatenate([wq, wk, wv], axis=0).T)
        in_maps.append({
            "xT": np.ascontiguousarray(x[b].T),
            "wqkvT": wqkvT,
            "woT": np.ascontiguousarray(w_out[:, sl].T),
            "masks": masks,
            "vones": np.ones((P, NTT, HPC, 1), dtype=np.float32),
        })
    return in_maps


def kernel(x, w_qkv, w_out, _trace=False):
    x = np.asarray(x, dtype=np.float32)
    w_qkv = np.asarray(w_qkv, dtype=np.float32)
    w_out = np.asarray(w_out, dtype=np.float32)

    if "nc" not in _CACHE:
        _CACHE["nc"] = build()
    nc = _CACHE["nc"]

    in_maps = _shard_inputs(x, w_qkv, w_out)
    res = bass_utils.run_bass_kernel_spmd(
        nc, in_maps, core_ids=list(range(8)), trace=_trace)
    kernel.last_result = res

    out = np.empty((B, T, E), dtype=np.float32)
    for b in range(B):
        out[b] = res.results[2 * b]["out"] + res.results[2 * b + 1]["out"]
    return out
